# revision 46
# baseline (speedup 1.0000x reference)
"""MHSA (cosine-sim attention) Trainium2 Bass kernel.

Shapes (hardcoded): x [2, 64, 64, 64] -> L=4096, 2 heads, dk=64, dv=32.
Sharding: 8 cores = (batch n, query-quarter). Each core computes K/V over the
full sequence for its n (redundant, cheap) and attention + merge for its own
1024 query columns. Outputs are disjoint -> no collectives.

All matmuls run in bf16 (fp32 PSUM accumulate). The fp32 residual dominates
the output magnitude, so bf16 attention internals keep overall rel-err ~1e-4.

Per-core dataflow:
  - projections with bias folded via ones-row on x (contract dim 65)
  - Q,K in [head*64+dk, L]; l2-norm: DVE square -> block-ones matmul (per-head
    sumsq broadcast to all 128 partitions) -> fast-reciprocal -> ACT sqrt ->
    elementwise multiply
  - V built directly transposed [key-chunk(128), 33] with a trailing ones
    column, so the A@V matmul also yields the softmax denominator (row 32)
  - flash-style loop per head: S^T chunk [128 keys, 1024 q] (2 matmuls) ->
    exp on ACT [128,1024] -> accumulate R via PE; R matmuls emitted one
    iteration late so the PE never waits on the ACT exp
  - epilogue: 1/s broadcast via tiny matmul into spare partitions of the same
    PSUM bank, scale R, merge both heads + bias, add fp32 residual, DMA out
"""

import os
import numpy as np
import ml_dtypes
from contextlib import ExitStack

DEBUG = os.environ.get("MHSA_DBG") == "1"

N, C = 2, 64
L = 64 * 64            # 4096
HEADS, DK, DV = 2, 64, 32
NCORES = 8
SHARDS_PER_N = NCORES // N   # 4
QSH = L // SHARDS_PER_N      # 1024
NB = 512                     # PSUM-bank-sized matmul free dim (fp32)
MC = 128                     # key chunk (PE output partitions)
BF16 = ml_dtypes.bfloat16

_CACHE = {}


def _build_program():
    import concourse.bass as bass
    import concourse.mybir as mybir
    import concourse.tile as tile
    from concourse import bacc
    from concourse.tile import add_dep_helper

    dt = mybir.dt
    AF = mybir.ActivationFunctionType
    nc = bacc.Bacc("TRN2", target_bir_lowering=False, debug=False,
                   num_devices=NCORES)

    # ---- DRAM I/O (per core; names are the in_map keys) ----
    xfb = nc.dram_tensor("xfb", [C + 1, L], dt.bfloat16, kind="ExternalInput")
    xqb = nc.dram_tensor("xqb", [C + 1, QSH], dt.bfloat16, kind="ExternalInput")
    xq32 = nc.dram_tensor("xq32", [C, QSH], dt.float32, kind="ExternalInput")
    wqT = nc.dram_tensor("wqT", [C + 1, HEADS * DK], dt.bfloat16, kind="ExternalInput")
    wkT = nc.dram_tensor("wkT", [C + 1, HEADS * DK], dt.bfloat16, kind="ExternalInput")
    wvT = nc.dram_tensor("wvT", [C + 1, HEADS * DV], dt.bfloat16, kind="ExternalInput")
    wm0 = nc.dram_tensor("wm0", [DV + 1, C], dt.bfloat16, kind="ExternalInput")
    wm1 = nc.dram_tensor("wm1", [DV, C], dt.bfloat16, kind="ExternalInput")
    blk = nc.dram_tensor("blk", [128, 128], dt.bfloat16, kind="ExternalInput")
    out = nc.dram_tensor("out", [C, QSH], dt.float32, kind="ExternalOutput")
    if DEBUG:
        dbg_q = nc.dram_tensor("dbg_q", [128, QSH], dt.bfloat16, kind="ExternalOutput")
        dbg_k = nc.dram_tensor("dbg_k", [128, L], dt.bfloat16, kind="ExternalOutput")
        dbg_vt = nc.dram_tensor("dbg_vt", [128, 32 * 66], dt.bfloat16, kind="ExternalOutput")
        dbg_et = nc.dram_tensor("dbg_et", [128, 2 * NB], dt.bfloat16, kind="ExternalOutput")
        dbg_rp = nc.dram_tensor("dbg_rp", [DV + 1, NB], dt.float32, kind="ExternalOutput")
        dbg_sbs = nc.dram_tensor("dbg_sbs", [DV, NB], dt.bfloat16, kind="ExternalOutput")
        dbg_rc = nc.dram_tensor("dbg_rc", [DV + 1, NB], dt.bfloat16, kind="ExternalOutput")

    KCH = L // NB        # 8 K-projection chunks
    QCH = QSH // NB      # 2 Q-projection chunks
    NMC = L // MC        # 32 key chunks
    LBS = QSH // NB      # 2 query blocks

    with tile.TileContext(nc) as tc, ExitStack() as ctx:
        per = ctx.enter_context(tc.tile_pool(name="per", bufs=1))
        rot = ctx.enter_context(tc.tile_pool(name="rot", bufs=3))
        etp = ctx.enter_context(tc.tile_pool(name="etp", bufs=4))
        # PSUM budget (8 banks): big 2x[128,1024]=4, rpe 2x[33,1024]=4
        # (rpe slots also host the epilogue sbc/op tiles via tag sharing)
        big = ctx.enter_context(tc.tile_pool(name="big", bufs=2, space="PSUM"))
        rpe = ctx.enter_context(tc.tile_pool(name="rpe", bufs=2, space="PSUM"))

        # ---- persistent SBUF tiles ----
        xfb_sb = per.tile([C + 1, L], dt.bfloat16, tag="xfb")
        xqb_sb = per.tile([C + 1, QSH], dt.bfloat16, tag="xqb")
        xq32_sb = per.tile([C, QSH], dt.float32, tag="xq32")
        wqT_sb = per.tile([C + 1, HEADS * DK], dt.bfloat16, tag="wqT")
        wkT_sb = per.tile([C + 1, HEADS * DK], dt.bfloat16, tag="wkT")
        wvT_sb = per.tile([C + 1, HEADS * DV], dt.bfloat16, tag="wvT")
        wm0_sb = per.tile([DV + 1, C], dt.bfloat16, tag="wm0")
        wm1_sb = per.tile([DV, C], dt.bfloat16, tag="wm1")
        blk_sb = per.tile([128, 128], dt.bfloat16, tag="blk")
        K_sb = per.tile([128, L], dt.bfloat16, tag="K")
        Q_sb = per.tile([128, QSH], dt.bfloat16, tag="Q")
        # V^T, per key chunk: [Vh0(32) | ones | Vh1(32) | ones] = 66 cols
        VT_sb = per.tile([128, NMC * 66], dt.bfloat16, tag="VT")

        # PE warm-up: ~5us of dependency-free matmuls while the input DMAs
        # run, so the HAM clock gate reaches 2.4 GHz before real work starts
        warm_sb = per.tile([128, NB], dt.bfloat16, tag="warm")
        nc.vector.memset(warm_sb[:], 0.01)
        for i in range(12):
            w = big.tile([128, NB], dt.float32, tag="big", name=f"warm{i}")
            nc.tensor.matmul(w[:], warm_sb[:, 0:128], warm_sb[:],
                             start=True, stop=True)

        nc.sync.dma_start(wqT_sb[:], wqT[:])
        nc.sync.dma_start(wkT_sb[:], wkT[:])
        nc.sync.dma_start(wvT_sb[:], wvT[:])
        nc.sync.dma_start(wm0_sb[:], wm0[:])
        nc.sync.dma_start(wm1_sb[:], wm1[:])
        nc.sync.dma_start(blk_sb[:], blk[:])
        for cb in range(QCH):
            sl = slice(cb * NB, (cb + 1) * NB)
            nc.sync.dma_start(xqb_sb[:, sl], xqb[:, sl])
        for cb in range(KCH):
            sl = slice(cb * NB, (cb + 1) * NB)
            nc.sync.dma_start(xfb_sb[:, sl], xfb[:, sl])
        nc.sync.dma_start(xq32_sb[:], xq32[:])

        # ones columns of VT (col 32 and 65 of each 66-block), one strided memset
        vt_ones = VT_sb[:, :].rearrange("p (m g c) -> p m g c", m=NMC, g=2)[:, :, :, 32:33]
        nc.vector.memset(vt_ones, 1.0)

        # ---- projections + l2 normalization for Q and K ----
        last_sqrt = None

        def proj_norm(dst_sb, w_sb, src_sb, nchunks):
            nonlocal last_sqrt
            for cb in range(nchunks):
                sl = slice(cb * NB, (cb + 1) * NB)
                pp = big.tile([128, NB], dt.float32, tag="big")
                nc.tensor.matmul(pp[:], w_sb[:], src_sb[:, sl], start=True, stop=True)
                nc.vector.tensor_copy(dst_sb[:, sl], pp[:])
                sq = rot.tile([128, NB], dt.bfloat16, tag="sq")
                nc.vector.tensor_mul(sq[:], dst_sb[:, sl], dst_sb[:, sl])
                ssq = big.tile([128, NB], dt.float32, tag="big")
                nc.tensor.matmul(ssq[:], blk_sb[:], sq[:], start=True, stop=True)
                # norm = sqrt(ssq) on ACT (PSUM ok), then 1/norm on DVE
                # (reciprocal_approx_fast wants SBUF fp32 in/out)
                nrm = rot.tile([128, NB], dt.float32, tag="nrm")
                last_sqrt = nc.scalar.activation(nrm[:], ssq[:], AF.Sqrt)
                rcp = rot.tile([128, NB], dt.float32, tag="rcp")
                nc.vector.reciprocal_approx_fast(rcp[:], nrm[:])
                inv = rot.tile([128, NB], dt.bfloat16, tag="inv")
                nc.vector.tensor_copy(inv[:], rcp[:])
                nc.vector.tensor_mul(dst_sb[:, sl], dst_sb[:, sl], inv[:])

        proj_norm(Q_sb, wqT_sb, xqb_sb, QCH)
        proj_norm(K_sb, wkT_sb, xfb_sb, KCH)

        # ---- V^T projection (both heads per 128-column chunk) ----
        for mc in range(NMC):
            vp = big.tile([128, HEADS * DV], dt.float32, tag="big")
            nc.tensor.matmul(
                vp[:], xfb_sb[:, mc * MC:(mc + 1) * MC], wvT_sb[:],
                start=True, stop=True,
            )
            dst = VT_sb[:, mc * 66:(mc + 1) * 66].rearrange(
                "p (g c) -> p g c", g=2)[:, :, 0:32]
            src = vp[:, :].rearrange("p (g c) -> p g c", g=2)
            nc.vector.tensor_copy(dst, src)

        # ---- attention: flash loop per head, software-pipelined ----
        rc_store = [[None] * LBS, [None] * LBS]
        first_exp = True
        for h in range(2):
            hsl = slice(h * DK, (h + 1) * DK)
            vsl = lambda m: slice(m * 66 + h * 33, m * 66 + h * 33 + 33)
            # rp rows 0-31: R accum; row 32: exp-sum (both query blocks wide)
            rp_acc = rpe.tile([DV + 1, 2 * NB], dt.float32, tag="rp",
                              name=f"rp{h}")
            pending = None
            for mc in range(NMC):
                sp = big.tile([128, 2 * NB], dt.float32, tag="big")
                for lb in range(LBS):
                    nc.tensor.matmul(
                        sp[:, lb * NB:(lb + 1) * NB],
                        K_sb[hsl, mc * MC:(mc + 1) * MC],
                        Q_sb[hsl, lb * NB:(lb + 1) * NB],
                        start=True, stop=True,
                    )
                et = etp.tile([128, 2 * NB], dt.bfloat16, tag="et")
                e = nc.scalar.activation(et[:], sp[:], AF.Exp)
                if DEBUG and h == 0 and mc == 0:
                    nc.sync.dma_start(dbg_et[:], et[:])
                if first_exp and last_sqrt is not None:
                    add_dep_helper(e.ins, last_sqrt.ins, sync=True,
                                   reason="keep exp table-set after all sqrts")
                    first_exp = False
                if pending is not None:
                    pmc, pet = pending
                    for lb in range(LBS):
                        nc.tensor.matmul(
                            rp_acc[:, lb * NB:(lb + 1) * NB],
                            VT_sb[:, vsl(pmc)],
                            pet[:, lb * NB:(lb + 1) * NB],
                            start=(pmc == 0), stop=False,
                        )
                pending = (mc, et)
            pmc, pet = pending
            for lb in range(LBS):
                nc.tensor.matmul(
                    rp_acc[:, lb * NB:(lb + 1) * NB],
                    VT_sb[:, vsl(pmc)],
                    pet[:, lb * NB:(lb + 1) * NB],
                    start=False, stop=True,
                )
            # epilogue per query block: scale R rows by 1/s.
            # Broadcast s across 32 partitions first (ones lhsT at part 32,
            # PSUM out at partition 0), then take the reciprocal on the
            # [32, NB] tile -- reciprocal_approx_fast mislowers on
            # single-partition slices on HW.
            for lb in range(LBS):
                lsl = slice(lb * NB, (lb + 1) * NB)
                rp = rp_acc[:, lsl]
                sb16 = rot.tile([DV + 1, NB], dt.bfloat16, tag="sb16")
                nc.vector.tensor_copy(sb16[DV:DV + 1, :], rp[DV:DV + 1, :])
                sbc = rpe.tile([DV, NB], dt.float32, tag="rp",
                               name=f"sbc{h}_{lb}")
                nc.tensor.matmul(sbc[:], blk_sb[DV:DV + 1, 0:DV],
                                 sb16[DV:DV + 1, :], start=True, stop=True)
                sfull = rot.tile([DV, NB], dt.float32, tag="sfull")
                nc.vector.tensor_copy(sfull[:], sbc[:])
                rinv = rot.tile([DV, NB], dt.float32, tag="rinv")
                nc.vector.reciprocal_approx_fast(rinv[:], sfull[:])
                sbs = rot.tile([DV, NB], dt.bfloat16, tag="sbs")
                nc.vector.tensor_copy(sbs[:], rinv[:])
                if DEBUG and h == 0 and lb == 0:
                    rpc = rot.tile([DV + 1, NB], dt.float32, tag="rpc")
                    nc.vector.tensor_copy(rpc[:], rp[:])
                    nc.sync.dma_start(dbg_rp[:], rpc[:])
                    nc.sync.dma_start(dbg_sbs[:], sbs[:])
                if h == 0:
                    rc = rot.tile([DV + 1, NB], dt.bfloat16, tag="rc0",
                                  name=f"rc0_{lb}")
                    nc.vector.memset(rc[DV:DV + 1, :], 1.0)
                else:
                    rc = rot.tile([DV, NB], dt.bfloat16, tag="rc1",
                                  name=f"rc1_{lb}")
                nc.vector.tensor_mul(rc[0:DV, :], rp[0:DV, :], sbs[:])
                if DEBUG and h == 0 and lb == 0:
                    nc.sync.dma_start(dbg_rc[:], rc[:])
                rc_store[h][lb] = rc
        if DEBUG:
            nc.sync.dma_start(dbg_q[:], Q_sb[:])
            nc.sync.dma_start(dbg_k[:], K_sb[:])
            nc.sync.dma_start(dbg_vt[:], VT_sb[:])
        for lb in range(LBS):
            lsl = slice(lb * NB, (lb + 1) * NB)
            op = rpe.tile([C, NB], dt.float32, tag="rp", name=f"op{lb}")
            nc.tensor.matmul(op[:], wm0_sb[:], rc_store[0][lb][:],
                             start=True, stop=False)
            nc.tensor.matmul(op[:], wm1_sb[:], rc_store[1][lb][:],
                             start=False, stop=True)
            o_sb = rot.tile([C, NB], dt.float32, tag="o")
            nc.vector.tensor_add(o_sb[:], op[:], xq32_sb[:, lsl])
            nc.sync.dma_start(out[:, lsl], o_sb[:])

    nc.compile()
    return nc


def get_program():
    if "nc" not in _CACHE:
        _CACHE["nc"] = _build_program()
    return _CACHE["nc"]


def make_in_maps(x, Wq, bq, Wk, bk, Wv, bv, Wm, bm):
    """Host-side layout prep + per-core sharding. Pure layout, no compute."""
    xf = np.asarray(x, np.float32).reshape(N, C, L)
    onesL = np.ones((1, L), np.float32)

    wqT = np.concatenate(
        [np.transpose(np.asarray(Wq, np.float32), (2, 0, 1)).reshape(C, HEADS * DK),
         np.asarray(bq, np.float32).reshape(1, HEADS * DK)], 0).astype(BF16)
    wkT = np.concatenate(
        [np.transpose(np.asarray(Wk, np.float32), (2, 0, 1)).reshape(C, HEADS * DK),
         np.asarray(bk, np.float32).reshape(1, HEADS * DK)], 0).astype(BF16)
    wvT = np.concatenate(
        [np.transpose(np.asarray(Wv, np.float32), (2, 0, 1)).reshape(C, HEADS * DV),
         np.asarray(bv, np.float32).reshape(1, HEADS * DV)], 0).astype(BF16)
    Wm32 = np.asarray(Wm, np.float32)
    wm0 = np.concatenate([Wm32[:, 0:DV].T,
                          np.asarray(bm, np.float32).reshape(1, C)], 0).astype(BF16)
    wm1 = Wm32[:, DV:2 * DV].T.astype(BF16)
    blk = np.zeros((128, 128), np.float32)
    blk[0:64, 0:64] = 1.0
    blk[64:128, 64:128] = 1.0
    blk = blk.astype(BF16)

    in_maps = []
    for core in range(NCORES):
        n, q = core // SHARDS_PER_N, core % SHARDS_PER_N
        qsl = slice(q * QSH, (q + 1) * QSH)
        xfb = np.concatenate([xf[n], onesL], 0).astype(BF16)
        xq = xf[n][:, qsl]
        xqb = np.concatenate([xq, np.ones((1, QSH), np.float32)], 0).astype(BF16)
        in_maps.append({
            "xfb": xfb, "xqb": xqb, "xq32": np.ascontiguousarray(xq),
            "wqT": wqT, "wkT": wkT, "wvT": wvT,
            "wm0": wm0, "wm1": wm1, "blk": blk,
        })
    return in_maps


def assemble(results):
    outf = np.empty((N, C, L), np.float32)
    for core in range(NCORES):
        n, q = core // SHARDS_PER_N, core % SHARDS_PER_N
        outf[n][:, q * QSH:(q + 1) * QSH] = results[core]["out"]
    return outf.reshape(N, C, 64, 64)


def run(inputs, trace=False, **kw):
    from concourse.bass_utils import run_bass_kernel_spmd
    nc = get_program()
    in_maps = make_in_maps(**inputs)
    res = run_bass_kernel_spmd(nc, in_maps, list(range(NCORES)), trace=trace, **kw)
    return assemble(res.results), res


def kernel(**inputs):
    out, _ = run(inputs)
    return out


# revision 48
# speedup vs baseline: 1.0819x; 1.0819x over previous
"""MHSA (cosine-sim attention) Trainium2 Bass kernel.

Shapes (hardcoded): x [2, 64, 64, 64] -> L=4096, 2 heads, dk=64, dv=32.
Sharding: 8 cores = (batch n, query-quarter). Each core computes K/V over the
full sequence for its n (redundant, cheap) and attention + merge for its own
1024 query columns. Outputs are disjoint -> no collectives.

All matmuls run in bf16 (fp32 PSUM accumulate). The fp32 residual dominates
the output magnitude, so bf16 attention internals keep overall rel-err ~1e-4.

Per-core dataflow:
  - projections with bias folded via ones-row on x (contract dim 65)
  - Q,K in [head*64+dk, L]; l2-norm: DVE square -> block-ones matmul (per-head
    sumsq broadcast to all 128 partitions) -> fast-reciprocal -> ACT sqrt ->
    elementwise multiply
  - V built directly transposed [key-chunk(128), 33] with a trailing ones
    column, so the A@V matmul also yields the softmax denominator (row 32)
  - flash-style loop per head: S^T chunk [128 keys, 1024 q] (2 matmuls) ->
    exp on ACT [128,1024] -> accumulate R via PE; R matmuls emitted one
    iteration late so the PE never waits on the ACT exp
  - epilogue: 1/s broadcast via tiny matmul into spare partitions of the same
    PSUM bank, scale R, merge both heads + bias, add fp32 residual, DMA out
"""

import os
import numpy as np
import ml_dtypes
from contextlib import ExitStack

DEBUG = os.environ.get("MHSA_DBG") == "1"

N, C = 2, 64
L = 64 * 64            # 4096
HEADS, DK, DV = 2, 64, 32
NCORES = 8
SHARDS_PER_N = NCORES // N   # 4
QSH = L // SHARDS_PER_N      # 1024
NB = 512                     # PSUM-bank-sized matmul free dim (fp32)
MC = 128                     # key chunk (PE output partitions)
BF16 = ml_dtypes.bfloat16

_CACHE = {}


def _build_program():
    import concourse.bass as bass
    import concourse.mybir as mybir
    import concourse.tile as tile
    from concourse import bacc
    from concourse.tile import add_dep_helper

    dt = mybir.dt
    AF = mybir.ActivationFunctionType
    nc = bacc.Bacc("TRN2", target_bir_lowering=False, debug=False,
                   num_devices=NCORES)

    # ---- DRAM I/O (per core; names are the in_map keys) ----
    xfb = nc.dram_tensor("xfb", [C + 1, L], dt.bfloat16, kind="ExternalInput")
    xqb = nc.dram_tensor("xqb", [C + 1, QSH], dt.bfloat16, kind="ExternalInput")
    xq32 = nc.dram_tensor("xq32", [C, QSH], dt.float32, kind="ExternalInput")
    wqT = nc.dram_tensor("wqT", [C + 1, HEADS * DK], dt.bfloat16, kind="ExternalInput")
    wkT = nc.dram_tensor("wkT", [C + 1, HEADS * DK], dt.bfloat16, kind="ExternalInput")
    wvT = nc.dram_tensor("wvT", [C + 1, HEADS * DV], dt.bfloat16, kind="ExternalInput")
    wm0 = nc.dram_tensor("wm0", [DV + 1, C], dt.bfloat16, kind="ExternalInput")
    wm1 = nc.dram_tensor("wm1", [DV, C], dt.bfloat16, kind="ExternalInput")
    blk = nc.dram_tensor("blk", [128, 128], dt.bfloat16, kind="ExternalInput")
    out = nc.dram_tensor("out", [C, QSH], dt.float32, kind="ExternalOutput")
    if DEBUG:
        dbg_q = nc.dram_tensor("dbg_q", [128, QSH], dt.bfloat16, kind="ExternalOutput")
        dbg_k = nc.dram_tensor("dbg_k", [128, L], dt.bfloat16, kind="ExternalOutput")
        dbg_vt = nc.dram_tensor("dbg_vt", [128, 32 * 66], dt.bfloat16, kind="ExternalOutput")
        dbg_et = nc.dram_tensor("dbg_et", [128, 2 * NB], dt.bfloat16, kind="ExternalOutput")
        dbg_rp = nc.dram_tensor("dbg_rp", [DV + 1, NB], dt.float32, kind="ExternalOutput")
        dbg_sbs = nc.dram_tensor("dbg_sbs", [DV, NB], dt.bfloat16, kind="ExternalOutput")
        dbg_rc = nc.dram_tensor("dbg_rc", [DV + 1, NB], dt.bfloat16, kind="ExternalOutput")

    KCH = L // NB        # 8 K-projection chunks
    QCH = QSH // NB      # 2 Q-projection chunks
    NMC = L // MC        # 32 key chunks
    LBS = QSH // NB      # 2 query blocks

    with tile.TileContext(nc) as tc, ExitStack() as ctx:
        per = ctx.enter_context(tc.tile_pool(name="per", bufs=1))
        rot = ctx.enter_context(tc.tile_pool(name="rot", bufs=3))
        etp = ctx.enter_context(tc.tile_pool(name="etp", bufs=4))
        # PSUM budget (8 banks): big 2x[128,1024]=4, rpe 2x[33,1024]=4
        # (rpe slots also host the epilogue sbc/op tiles via tag sharing)
        big = ctx.enter_context(tc.tile_pool(name="big", bufs=2, space="PSUM"))
        rpe = ctx.enter_context(tc.tile_pool(name="rpe", bufs=2, space="PSUM"))

        # ---- persistent SBUF tiles ----
        xfb_sb = per.tile([C + 1, L], dt.bfloat16, tag="xfb")
        xqb_sb = per.tile([C + 1, QSH], dt.bfloat16, tag="xqb")
        xq32_sb = per.tile([C, QSH], dt.float32, tag="xq32")
        wqT_sb = per.tile([C + 1, HEADS * DK], dt.bfloat16, tag="wqT")
        wkT_sb = per.tile([C + 1, HEADS * DK], dt.bfloat16, tag="wkT")
        wvT_sb = per.tile([C + 1, HEADS * DV], dt.bfloat16, tag="wvT")
        wm0_sb = per.tile([DV + 1, C], dt.bfloat16, tag="wm0")
        wm1_sb = per.tile([DV, C], dt.bfloat16, tag="wm1")
        blk_sb = per.tile([128, 128], dt.bfloat16, tag="blk")
        K_sb = per.tile([128, L], dt.bfloat16, tag="K")
        Q_sb = per.tile([128, QSH], dt.bfloat16, tag="Q")
        # V^T, per key chunk: [Vh0(32) | ones | Vh1(32) | ones] = 66 cols
        VT_sb = per.tile([128, NMC * 66], dt.bfloat16, tag="VT")

        # PE warm-up: ~5us of dependency-free matmuls while the input DMAs
        # run, so the HAM clock gate reaches 2.4 GHz before real work starts
        warm_sb = per.tile([128, NB], dt.bfloat16, tag="warm")
        nc.vector.memset(warm_sb[:], 0.01)
        for i in range(12):
            w = big.tile([128, NB], dt.float32, tag="big", name=f"warm{i}")
            nc.tensor.matmul(w[:], warm_sb[:, 0:128], warm_sb[:],
                             start=True, stop=True)

        nc.sync.dma_start(wqT_sb[:], wqT[:])
        nc.sync.dma_start(wkT_sb[:], wkT[:])
        nc.sync.dma_start(wvT_sb[:], wvT[:])
        nc.sync.dma_start(wm0_sb[:], wm0[:])
        nc.sync.dma_start(wm1_sb[:], wm1[:])
        nc.sync.dma_start(blk_sb[:], blk[:])
        for cb in range(QCH):
            sl = slice(cb * NB, (cb + 1) * NB)
            nc.sync.dma_start(xqb_sb[:, sl], xqb[:, sl])
        for cb in range(KCH):
            sl = slice(cb * NB, (cb + 1) * NB)
            nc.sync.dma_start(xfb_sb[:, sl], xfb[:, sl])
        nc.sync.dma_start(xq32_sb[:], xq32[:])

        # ones columns of VT (col 32 and 65 of each 66-block), one strided memset
        vt_ones = VT_sb[:, :].rearrange("p (m g c) -> p m g c", m=NMC, g=2)[:, :, :, 32:33]
        nc.vector.memset(vt_ones, 1.0)

        # ---- projections; Q gets explicit l2 norm, K's 1/||k|| is folded
        # into the exp's per-partition scale (S^T partitions are keys) ----
        last_sqrt = None
        # head-indicator columns {0, 64} of blk: [128, 2] with ind[p,h]=1
        # iff p belongs to head h
        ind2 = blk_sb[:, :].rearrange("p (a b) -> p a b", a=2)[:, :, 0:1]

        def proj_norm_q():
            nonlocal last_sqrt
            for cb in range(QCH):
                sl = slice(cb * NB, (cb + 1) * NB)
                pp = big.tile([128, NB], dt.float32, tag="big")
                nc.tensor.matmul(pp[:], wqT_sb[:], xqb_sb[:, sl],
                                 start=True, stop=True)
                nc.vector.tensor_copy(Q_sb[:, sl], pp[:])
                sq = rot.tile([128, NB], dt.bfloat16, tag="sq")
                nc.vector.tensor_mul(sq[:], Q_sb[:, sl], Q_sb[:, sl])
                ssq = big.tile([128, NB], dt.float32, tag="big")
                nc.tensor.matmul(ssq[:], blk_sb[:], sq[:], start=True, stop=True)
                nrm = rot.tile([128, NB], dt.float32, tag="nrm")
                last_sqrt = nc.scalar.activation(nrm[:], ssq[:], AF.Sqrt)
                rcp = rot.tile([128, NB], dt.float32, tag="rcp")
                nc.vector.reciprocal_approx_fast(rcp[:], nrm[:])
                inv = rot.tile([128, NB], dt.bfloat16, tag="inv")
                nc.vector.tensor_copy(inv[:], rcp[:])
                nc.vector.tensor_mul(Q_sb[:, sl], Q_sb[:, sl], inv[:])

        # per-key 1/||k|| for both heads: [128 keys-of-chunk, 2*chunk] cols
        invKT_sb = per.tile([128, 2 * NMC], dt.float32, tag="invKT")

        def proj_k():
            nonlocal last_sqrt
            kssq = rpe.tile([128, 2 * NMC], dt.float32, tag="rp", name="kssq")
            for cb in range(KCH):
                sl = slice(cb * NB, (cb + 1) * NB)
                pp = big.tile([128, NB], dt.float32, tag="big")
                nc.tensor.matmul(pp[:], wkT_sb[:], xfb_sb[:, sl],
                                 start=True, stop=True)
                nc.vector.tensor_copy(K_sb[:, sl], pp[:])
                sq = rot.tile([128, NB], dt.bfloat16, tag="sq")
                nc.vector.tensor_mul(sq[:], K_sb[:, sl], K_sb[:, sl])
                for i in range(NB // MC):
                    mci = cb * (NB // MC) + i
                    nc.tensor.matmul(
                        kssq[:, 2 * mci:2 * mci + 2],
                        sq[:, i * MC:(i + 1) * MC], ind2,
                        start=True, stop=True,
                    )
            nrmT = rot.tile([128, 2 * NMC], dt.float32, tag="nrmT")
            last_sqrt = nc.scalar.activation(nrmT[:], kssq[:], AF.Sqrt)
            nc.vector.reciprocal_approx_fast(invKT_sb[:], nrmT[:])

        proj_norm_q()
        proj_k()

        # ---- V^T projection (both heads per 128-column chunk) ----
        for mc in range(NMC):
            vp = big.tile([128, HEADS * DV], dt.float32, tag="big")
            nc.tensor.matmul(
                vp[:], xfb_sb[:, mc * MC:(mc + 1) * MC], wvT_sb[:],
                start=True, stop=True,
            )
            dst = VT_sb[:, mc * 66:(mc + 1) * 66].rearrange(
                "p (g c) -> p g c", g=2)[:, :, 0:32]
            src = vp[:, :].rearrange("p (g c) -> p g c", g=2)
            nc.vector.tensor_copy(dst, src)

        # ---- attention: flash loop per head, software-pipelined ----
        rc_store = [[None] * LBS, [None] * LBS]
        first_exp = True
        for h in range(2):
            hsl = slice(h * DK, (h + 1) * DK)
            vsl = lambda m: slice(m * 66 + h * 33, m * 66 + h * 33 + 33)
            # rp rows 0-31: R accum; row 32: exp-sum (both query blocks wide)
            rp_acc = rpe.tile([DV + 1, 2 * NB], dt.float32, tag="rp",
                              name=f"rp{h}")
            pending = None
            for mc in range(NMC):
                sp = big.tile([128, 2 * NB], dt.float32, tag="big")
                for lb in range(LBS):
                    nc.tensor.matmul(
                        sp[:, lb * NB:(lb + 1) * NB],
                        K_sb[hsl, mc * MC:(mc + 1) * MC],
                        Q_sb[hsl, lb * NB:(lb + 1) * NB],
                        start=True, stop=True,
                    )
                et = etp.tile([128, 2 * NB], dt.bfloat16, tag="et")
                e = nc.scalar.activation(
                    et[:], sp[:], AF.Exp,
                    scale=invKT_sb[:, 2 * mc + h:2 * mc + h + 1])
                if DEBUG and h == 0 and mc == 0:
                    nc.sync.dma_start(dbg_et[:], et[:])
                if first_exp and last_sqrt is not None:
                    add_dep_helper(e.ins, last_sqrt.ins, sync=True,
                                   reason="keep exp table-set after all sqrts")
                    first_exp = False
                if pending is not None:
                    pmc, pet = pending
                    for lb in range(LBS):
                        nc.tensor.matmul(
                            rp_acc[:, lb * NB:(lb + 1) * NB],
                            VT_sb[:, vsl(pmc)],
                            pet[:, lb * NB:(lb + 1) * NB],
                            start=(pmc == 0), stop=False,
                        )
                pending = (mc, et)
            pmc, pet = pending
            for lb in range(LBS):
                nc.tensor.matmul(
                    rp_acc[:, lb * NB:(lb + 1) * NB],
                    VT_sb[:, vsl(pmc)],
                    pet[:, lb * NB:(lb + 1) * NB],
                    start=False, stop=True,
                )
            # epilogue per query block: scale R rows by 1/s.
            # Broadcast s across 32 partitions first (ones lhsT at part 32,
            # PSUM out at partition 0), then take the reciprocal on the
            # [32, NB] tile -- reciprocal_approx_fast mislowers on
            # single-partition slices on HW.
            for lb in range(LBS):
                lsl = slice(lb * NB, (lb + 1) * NB)
                rp = rp_acc[:, lsl]
                sb16 = rot.tile([DV + 1, NB], dt.bfloat16, tag="sb16")
                nc.vector.tensor_copy(sb16[DV:DV + 1, :], rp[DV:DV + 1, :])
                sbc = rpe.tile([DV, NB], dt.float32, tag="rp",
                               name=f"sbc{h}_{lb}")
                nc.tensor.matmul(sbc[:], blk_sb[DV:DV + 1, 0:DV],
                                 sb16[DV:DV + 1, :], start=True, stop=True)
                sfull = rot.tile([DV, NB], dt.float32, tag="sfull")
                nc.vector.tensor_copy(sfull[:], sbc[:])
                rinv = rot.tile([DV, NB], dt.float32, tag="rinv")
                nc.vector.reciprocal_approx_fast(rinv[:], sfull[:])
                sbs = rot.tile([DV, NB], dt.bfloat16, tag="sbs")
                nc.vector.tensor_copy(sbs[:], rinv[:])
                if DEBUG and h == 0 and lb == 0:
                    rpc = rot.tile([DV + 1, NB], dt.float32, tag="rpc")
                    nc.vector.tensor_copy(rpc[:], rp[:])
                    nc.sync.dma_start(dbg_rp[:], rpc[:])
                    nc.sync.dma_start(dbg_sbs[:], sbs[:])
                if h == 0:
                    rc = rot.tile([DV + 1, NB], dt.bfloat16, tag="rc0",
                                  name=f"rc0_{lb}")
                    nc.vector.memset(rc[DV:DV + 1, :], 1.0)
                else:
                    rc = rot.tile([DV, NB], dt.bfloat16, tag="rc1",
                                  name=f"rc1_{lb}")
                nc.vector.tensor_mul(rc[0:DV, :], rp[0:DV, :], sbs[:])
                if DEBUG and h == 0 and lb == 0:
                    nc.sync.dma_start(dbg_rc[:], rc[:])
                rc_store[h][lb] = rc
        if DEBUG:
            nc.sync.dma_start(dbg_q[:], Q_sb[:])
            nc.sync.dma_start(dbg_k[:], K_sb[:])
            nc.sync.dma_start(dbg_vt[:], VT_sb[:])
        for lb in range(LBS):
            lsl = slice(lb * NB, (lb + 1) * NB)
            op = rpe.tile([C, NB], dt.float32, tag="rp", name=f"op{lb}")
            nc.tensor.matmul(op[:], wm0_sb[:], rc_store[0][lb][:],
                             start=True, stop=False)
            nc.tensor.matmul(op[:], wm1_sb[:], rc_store[1][lb][:],
                             start=False, stop=True)
            o_sb = rot.tile([C, NB], dt.float32, tag="o")
            nc.vector.tensor_add(o_sb[:], op[:], xq32_sb[:, lsl])
            nc.sync.dma_start(out[:, lsl], o_sb[:])

    nc.compile()
    return nc


def get_program():
    if "nc" not in _CACHE:
        _CACHE["nc"] = _build_program()
    return _CACHE["nc"]


def make_in_maps(x, Wq, bq, Wk, bk, Wv, bv, Wm, bm):
    """Host-side layout prep + per-core sharding. Pure layout, no compute."""
    xf = np.asarray(x, np.float32).reshape(N, C, L)
    onesL = np.ones((1, L), np.float32)

    wqT = np.concatenate(
        [np.transpose(np.asarray(Wq, np.float32), (2, 0, 1)).reshape(C, HEADS * DK),
         np.asarray(bq, np.float32).reshape(1, HEADS * DK)], 0).astype(BF16)
    wkT = np.concatenate(
        [np.transpose(np.asarray(Wk, np.float32), (2, 0, 1)).reshape(C, HEADS * DK),
         np.asarray(bk, np.float32).reshape(1, HEADS * DK)], 0).astype(BF16)
    wvT = np.concatenate(
        [np.transpose(np.asarray(Wv, np.float32), (2, 0, 1)).reshape(C, HEADS * DV),
         np.asarray(bv, np.float32).reshape(1, HEADS * DV)], 0).astype(BF16)
    Wm32 = np.asarray(Wm, np.float32)
    wm0 = np.concatenate([Wm32[:, 0:DV].T,
                          np.asarray(bm, np.float32).reshape(1, C)], 0).astype(BF16)
    wm1 = Wm32[:, DV:2 * DV].T.astype(BF16)
    blk = np.zeros((128, 128), np.float32)
    blk[0:64, 0:64] = 1.0
    blk[64:128, 64:128] = 1.0
    blk = blk.astype(BF16)

    in_maps = []
    for core in range(NCORES):
        n, q = core // SHARDS_PER_N, core % SHARDS_PER_N
        qsl = slice(q * QSH, (q + 1) * QSH)
        xfb = np.concatenate([xf[n], onesL], 0).astype(BF16)
        xq = xf[n][:, qsl]
        xqb = np.concatenate([xq, np.ones((1, QSH), np.float32)], 0).astype(BF16)
        in_maps.append({
            "xfb": xfb, "xqb": xqb, "xq32": np.ascontiguousarray(xq),
            "wqT": wqT, "wkT": wkT, "wvT": wvT,
            "wm0": wm0, "wm1": wm1, "blk": blk,
        })
    return in_maps


def assemble(results):
    outf = np.empty((N, C, L), np.float32)
    for core in range(NCORES):
        n, q = core // SHARDS_PER_N, core % SHARDS_PER_N
        outf[n][:, q * QSH:(q + 1) * QSH] = results[core]["out"]
    return outf.reshape(N, C, 64, 64)


def run(inputs, trace=False, **kw):
    from concourse.bass_utils import run_bass_kernel_spmd
    nc = get_program()
    in_maps = make_in_maps(**inputs)
    res = run_bass_kernel_spmd(nc, in_maps, list(range(NCORES)), trace=trace, **kw)
    return assemble(res.results), res


def kernel(**inputs):
    out, _ = run(inputs)
    return out


# revision 51
# speedup vs baseline: 1.1109x; 1.0268x over previous
"""MHSA (cosine-sim attention) Trainium2 Bass kernel.

Shapes (hardcoded): x [2, 64, 64, 64] -> L=4096, 2 heads, dk=64, dv=32.
Sharding: 8 cores = (batch n, query-quarter). Each core computes K/V over the
full sequence for its n (redundant, cheap) and attention + merge for its own
1024 query columns. Outputs are disjoint -> no collectives.

All matmuls run in bf16 (fp32 PSUM accumulate). The fp32 residual dominates
the output magnitude, so bf16 attention internals keep overall rel-err ~1e-4.

Per-core dataflow:
  - projections with bias folded via ones-row on x (contract dim 65)
  - Q,K in [head*64+dk, L]; l2-norm: DVE square -> block-ones matmul (per-head
    sumsq broadcast to all 128 partitions) -> fast-reciprocal -> ACT sqrt ->
    elementwise multiply
  - V built directly transposed [key-chunk(128), 33] with a trailing ones
    column, so the A@V matmul also yields the softmax denominator (row 32)
  - flash-style loop per head: S^T chunk [128 keys, 1024 q] (2 matmuls) ->
    exp on ACT [128,1024] -> accumulate R via PE; R matmuls emitted one
    iteration late so the PE never waits on the ACT exp
  - epilogue: 1/s broadcast via tiny matmul into spare partitions of the same
    PSUM bank, scale R, merge both heads + bias, add fp32 residual, DMA out
"""

import os
import numpy as np
import ml_dtypes
from contextlib import ExitStack

DEBUG = os.environ.get("MHSA_DBG") == "1"

N, C = 2, 64
L = 64 * 64            # 4096
HEADS, DK, DV = 2, 64, 32
NCORES = 8
SHARDS_PER_N = NCORES // N   # 4
QSH = L // SHARDS_PER_N      # 1024
NB = 512                     # PSUM-bank-sized matmul free dim (fp32)
MC = 128                     # key chunk (PE output partitions)
BF16 = ml_dtypes.bfloat16

_CACHE = {}


def _build_program():
    import concourse.bass as bass
    import concourse.mybir as mybir
    import concourse.tile as tile
    from concourse import bacc
    from concourse.tile import add_dep_helper

    dt = mybir.dt
    AF = mybir.ActivationFunctionType
    nc = bacc.Bacc("TRN2", target_bir_lowering=False, debug=False,
                   num_devices=NCORES)

    # ---- DRAM I/O (per core; names are the in_map keys) ----
    xfb = nc.dram_tensor("xfb", [C + 1, L], dt.bfloat16, kind="ExternalInput")
    xqb = nc.dram_tensor("xqb", [C + 1, QSH], dt.bfloat16, kind="ExternalInput")
    xq32 = nc.dram_tensor("xq32", [C, QSH], dt.float32, kind="ExternalInput")
    wqT = nc.dram_tensor("wqT", [C + 1, HEADS * DK], dt.bfloat16, kind="ExternalInput")
    wkT = nc.dram_tensor("wkT", [C + 1, HEADS * DK], dt.bfloat16, kind="ExternalInput")
    wvT = nc.dram_tensor("wvT", [C + 1, HEADS * DV], dt.bfloat16, kind="ExternalInput")
    wm0 = nc.dram_tensor("wm0", [DV + 1, C], dt.bfloat16, kind="ExternalInput")
    wm1 = nc.dram_tensor("wm1", [DV, C], dt.bfloat16, kind="ExternalInput")
    blk = nc.dram_tensor("blk", [128, 128], dt.bfloat16, kind="ExternalInput")
    out = nc.dram_tensor("out", [C, QSH], dt.float32, kind="ExternalOutput")
    if DEBUG:
        dbg_q = nc.dram_tensor("dbg_q", [128, QSH], dt.bfloat16, kind="ExternalOutput")
        dbg_k = nc.dram_tensor("dbg_k", [128, L], dt.bfloat16, kind="ExternalOutput")
        dbg_vt = nc.dram_tensor("dbg_vt", [128, 32 * 66], dt.bfloat16, kind="ExternalOutput")
        dbg_et = nc.dram_tensor("dbg_et", [128, 2 * NB], dt.bfloat16, kind="ExternalOutput")
        dbg_rp = nc.dram_tensor("dbg_rp", [DV + 1, NB], dt.float32, kind="ExternalOutput")
        dbg_sbs = nc.dram_tensor("dbg_sbs", [DV, NB], dt.bfloat16, kind="ExternalOutput")
        dbg_rc = nc.dram_tensor("dbg_rc", [DV + 1, NB], dt.bfloat16, kind="ExternalOutput")

    KCH = L // NB        # 8 K-projection chunks
    QCH = QSH // NB      # 2 Q-projection chunks
    NMC = L // MC        # 32 key chunks
    LBS = QSH // NB      # 2 query blocks

    with tile.TileContext(nc) as tc, ExitStack() as ctx:
        per = ctx.enter_context(tc.tile_pool(name="per", bufs=1))
        rot = ctx.enter_context(tc.tile_pool(name="rot", bufs=3))
        etp = ctx.enter_context(tc.tile_pool(name="etp", bufs=6))
        # PSUM budget (8 banks): big 2x[128,1024]=4, rpe 2x[33,1024]=4
        # (rpe slots also host the epilogue sbc/op tiles via tag sharing)
        big = ctx.enter_context(tc.tile_pool(name="big", bufs=2, space="PSUM"))
        rpe = ctx.enter_context(tc.tile_pool(name="rpe", bufs=2, space="PSUM"))

        # ---- persistent SBUF tiles ----
        xfb_sb = per.tile([C + 1, L], dt.bfloat16, tag="xfb")
        xqb_sb = per.tile([C + 1, QSH], dt.bfloat16, tag="xqb")
        xq32_sb = per.tile([C, QSH], dt.float32, tag="xq32")
        wqT_sb = per.tile([C + 1, HEADS * DK], dt.bfloat16, tag="wqT")
        wkT_sb = per.tile([C + 1, HEADS * DK], dt.bfloat16, tag="wkT")
        wvT_sb = per.tile([C + 1, HEADS * DV], dt.bfloat16, tag="wvT")
        wm0_sb = per.tile([DV + 1, C], dt.bfloat16, tag="wm0")
        wm1_sb = per.tile([DV, C], dt.bfloat16, tag="wm1")
        blk_sb = per.tile([128, 128], dt.bfloat16, tag="blk")
        K_sb = per.tile([128, L], dt.bfloat16, tag="K")
        Q_sb = per.tile([128, QSH], dt.bfloat16, tag="Q")
        # V^T, per key chunk: [Vh0(32) | ones | Vh1(32) | ones] = 66 cols
        VT_sb = per.tile([128, NMC * 66], dt.bfloat16, tag="VT")

        # PE warm-up: ~5us of dependency-free matmuls while the input DMAs
        # run, so the HAM clock gate reaches 2.4 GHz before real work starts
        warm_sb = per.tile([128, NB], dt.bfloat16, tag="warm")
        nc.vector.memset(warm_sb[:], 0.01)
        for i in range(12):
            w = big.tile([128, NB], dt.float32, tag="big", name=f"warm{i}")
            nc.tensor.matmul(w[:], warm_sb[:, 0:128], warm_sb[:],
                             start=True, stop=True)

        nc.sync.dma_start(wqT_sb[:], wqT[:])
        nc.sync.dma_start(wkT_sb[:], wkT[:])
        nc.sync.dma_start(wvT_sb[:], wvT[:])
        nc.sync.dma_start(wm0_sb[:], wm0[:])
        nc.sync.dma_start(wm1_sb[:], wm1[:])
        nc.sync.dma_start(blk_sb[:], blk[:])
        for cb in range(QCH):
            sl = slice(cb * NB, (cb + 1) * NB)
            nc.sync.dma_start(xqb_sb[:, sl], xqb[:, sl])
        for cb in range(KCH):
            sl = slice(cb * NB, (cb + 1) * NB)
            nc.sync.dma_start(xfb_sb[:, sl], xfb[:, sl])
        nc.sync.dma_start(xq32_sb[:], xq32[:])

        # ones columns of VT (col 32 and 65 of each 66-block), one strided memset
        vt_ones = VT_sb[:, :].rearrange("p (m g c) -> p m g c", m=NMC, g=2)[:, :, :, 32:33]
        nc.vector.memset(vt_ones, 1.0)

        # ---- projections; Q gets explicit l2 norm, K's 1/||k|| is folded
        # into the exp's per-partition scale (S^T partitions are keys) ----
        last_sqrt = None
        # head-indicator columns {0, 64} of blk: [128, 2] with ind[p,h]=1
        # iff p belongs to head h
        ind2 = blk_sb[:, :].rearrange("p (a b) -> p a b", a=2)[:, :, 0:1]

        def proj_norm_q():
            nonlocal last_sqrt
            for cb in range(QCH):
                sl = slice(cb * NB, (cb + 1) * NB)
                pp = big.tile([128, NB], dt.float32, tag="big")
                nc.tensor.matmul(pp[:], wqT_sb[:], xqb_sb[:, sl],
                                 start=True, stop=True)
                nc.vector.tensor_copy(Q_sb[:, sl], pp[:])
                sq = rot.tile([128, NB], dt.bfloat16, tag="sq")
                nc.vector.tensor_mul(sq[:], Q_sb[:, sl], Q_sb[:, sl])
                ssq = big.tile([128, NB], dt.float32, tag="big")
                nc.tensor.matmul(ssq[:], blk_sb[:], sq[:], start=True, stop=True)
                nrm = rot.tile([128, NB], dt.float32, tag="nrm")
                last_sqrt = nc.scalar.activation(nrm[:], ssq[:], AF.Sqrt)
                rcp = rot.tile([128, NB], dt.float32, tag="rcp")
                nc.vector.reciprocal_approx_fast(rcp[:], nrm[:])
                inv = rot.tile([128, NB], dt.bfloat16, tag="inv")
                nc.vector.tensor_copy(inv[:], rcp[:])
                nc.vector.tensor_mul(Q_sb[:, sl], Q_sb[:, sl], inv[:])

        # per-key 1/||k|| for both heads: [128 keys-of-chunk, 2*chunk] cols
        invKT_sb = per.tile([128, 2 * NMC], dt.float32, tag="invKT")

        def proj_k():
            nonlocal last_sqrt
            kssq = rpe.tile([128, 2 * NMC], dt.float32, tag="rp", name="kssq")
            for cb in range(KCH):
                sl = slice(cb * NB, (cb + 1) * NB)
                pp = big.tile([128, NB], dt.float32, tag="big")
                nc.tensor.matmul(pp[:], wkT_sb[:], xfb_sb[:, sl],
                                 start=True, stop=True)
                nc.vector.tensor_copy(K_sb[:, sl], pp[:])
                sq = rot.tile([128, NB], dt.bfloat16, tag="sq")
                nc.vector.tensor_mul(sq[:], K_sb[:, sl], K_sb[:, sl])
                for i in range(NB // MC):
                    mci = cb * (NB // MC) + i
                    nc.tensor.matmul(
                        kssq[:, 2 * mci:2 * mci + 2],
                        sq[:, i * MC:(i + 1) * MC], ind2,
                        start=True, stop=True,
                    )
            nrmT = rot.tile([128, 2 * NMC], dt.float32, tag="nrmT")
            last_sqrt = nc.scalar.activation(nrmT[:], kssq[:], AF.Sqrt)
            nc.vector.reciprocal_approx_fast(invKT_sb[:], nrmT[:])

        proj_norm_q()
        proj_k()

        # ---- V^T projection (both heads per 128-column chunk) ----
        for mc in range(NMC):
            vp = big.tile([128, HEADS * DV], dt.float32, tag="big")
            nc.tensor.matmul(
                vp[:], xfb_sb[:, mc * MC:(mc + 1) * MC], wvT_sb[:],
                start=True, stop=True,
            )
            dst = VT_sb[:, mc * 66:(mc + 1) * 66].rearrange(
                "p (g c) -> p g c", g=2)[:, :, 0:32]
            src = vp[:, :].rearrange("p (g c) -> p g c", g=2)
            nc.vector.tensor_copy(dst, src)

        # ---- attention: flash loop per head, software-pipelined ----
        rc_store = [[None] * LBS, [None] * LBS]
        first_exp = True
        for h in range(2):
            hsl = slice(h * DK, (h + 1) * DK)
            vsl = lambda m: slice(m * 66 + h * 33, m * 66 + h * 33 + 33)
            # rp rows 0-31: R accum; row 32: exp-sum (both query blocks wide)
            rp_acc = rpe.tile([DV + 1, 2 * NB], dt.float32, tag="rp",
                              name=f"rp{h}")
            pending = []
            for mc in range(NMC):
                sp = big.tile([128, 2 * NB], dt.float32, tag="big")
                for lb in range(LBS):
                    nc.tensor.matmul(
                        sp[:, lb * NB:(lb + 1) * NB],
                        K_sb[hsl, mc * MC:(mc + 1) * MC],
                        Q_sb[hsl, lb * NB:(lb + 1) * NB],
                        start=True, stop=True,
                    )
                et = etp.tile([128, 2 * NB], dt.bfloat16, tag="et")
                e = nc.scalar.activation(
                    et[:], sp[:], AF.Exp,
                    scale=invKT_sb[:, 2 * mc + h:2 * mc + h + 1])
                if DEBUG and h == 0 and mc == 0:
                    nc.sync.dma_start(dbg_et[:], et[:])
                if first_exp and last_sqrt is not None:
                    add_dep_helper(e.ins, last_sqrt.ins, sync=True,
                                   reason="keep exp table-set after all sqrts")
                    first_exp = False
                pending.append((mc, et))
                if len(pending) > 2:
                    pmc, pet = pending.pop(0)
                    for lb in range(LBS):
                        nc.tensor.matmul(
                            rp_acc[:, lb * NB:(lb + 1) * NB],
                            VT_sb[:, vsl(pmc)],
                            pet[:, lb * NB:(lb + 1) * NB],
                            start=(pmc == 0), stop=False,
                        )
            for pmc, pet in pending:
                for lb in range(LBS):
                    nc.tensor.matmul(
                        rp_acc[:, lb * NB:(lb + 1) * NB],
                        VT_sb[:, vsl(pmc)],
                        pet[:, lb * NB:(lb + 1) * NB],
                        start=(pmc == 0), stop=(pmc == NMC - 1),
                    )
            # epilogue per query block: scale R rows by 1/s.
            # Broadcast s across 32 partitions first (ones lhsT at part 32,
            # PSUM out at partition 0), then take the reciprocal on the
            # [32, NB] tile -- reciprocal_approx_fast mislowers on
            # single-partition slices on HW.
            for lb in range(LBS):
                lsl = slice(lb * NB, (lb + 1) * NB)
                rp = rp_acc[:, lsl]
                sb16 = rot.tile([DV + 1, NB], dt.bfloat16, tag="sb16")
                nc.vector.tensor_copy(sb16[DV:DV + 1, :], rp[DV:DV + 1, :])
                sbc = rpe.tile([DV, NB], dt.float32, tag="rp",
                               name=f"sbc{h}_{lb}")
                nc.tensor.matmul(sbc[:], blk_sb[DV:DV + 1, 0:DV],
                                 sb16[DV:DV + 1, :], start=True, stop=True)
                sfull = rot.tile([DV, NB], dt.float32, tag="sfull")
                nc.vector.tensor_copy(sfull[:], sbc[:])
                rinv = rot.tile([DV, NB], dt.float32, tag="rinv")
                nc.vector.reciprocal_approx_fast(rinv[:], sfull[:])
                sbs = rot.tile([DV, NB], dt.bfloat16, tag="sbs")
                nc.vector.tensor_copy(sbs[:], rinv[:])
                if DEBUG and h == 0 and lb == 0:
                    rpc = rot.tile([DV + 1, NB], dt.float32, tag="rpc")
                    nc.vector.tensor_copy(rpc[:], rp[:])
                    nc.sync.dma_start(dbg_rp[:], rpc[:])
                    nc.sync.dma_start(dbg_sbs[:], sbs[:])
                if h == 0:
                    rc = rot.tile([DV + 1, NB], dt.bfloat16, tag="rc0",
                                  name=f"rc0_{lb}")
                    nc.vector.memset(rc[DV:DV + 1, :], 1.0)
                else:
                    rc = rot.tile([DV, NB], dt.bfloat16, tag="rc1",
                                  name=f"rc1_{lb}")
                nc.vector.tensor_mul(rc[0:DV, :], rp[0:DV, :], sbs[:])
                if DEBUG and h == 0 and lb == 0:
                    nc.sync.dma_start(dbg_rc[:], rc[:])
                rc_store[h][lb] = rc
        if DEBUG:
            nc.sync.dma_start(dbg_q[:], Q_sb[:])
            nc.sync.dma_start(dbg_k[:], K_sb[:])
            nc.sync.dma_start(dbg_vt[:], VT_sb[:])
        for lb in range(LBS):
            lsl = slice(lb * NB, (lb + 1) * NB)
            op = rpe.tile([C, NB], dt.float32, tag="rp", name=f"op{lb}")
            nc.tensor.matmul(op[:], wm0_sb[:], rc_store[0][lb][:],
                             start=True, stop=False)
            nc.tensor.matmul(op[:], wm1_sb[:], rc_store[1][lb][:],
                             start=False, stop=True)
            o_sb = rot.tile([C, NB], dt.float32, tag="o")
            nc.vector.tensor_add(o_sb[:], op[:], xq32_sb[:, lsl])
            nc.sync.dma_start(out[:, lsl], o_sb[:])

    nc.compile()
    return nc


def get_program():
    if "nc" not in _CACHE:
        _CACHE["nc"] = _build_program()
    return _CACHE["nc"]


def make_in_maps(x, Wq, bq, Wk, bk, Wv, bv, Wm, bm):
    """Host-side layout prep + per-core sharding. Pure layout, no compute."""
    xf = np.asarray(x, np.float32).reshape(N, C, L)
    onesL = np.ones((1, L), np.float32)

    wqT = np.concatenate(
        [np.transpose(np.asarray(Wq, np.float32), (2, 0, 1)).reshape(C, HEADS * DK),
         np.asarray(bq, np.float32).reshape(1, HEADS * DK)], 0).astype(BF16)
    wkT = np.concatenate(
        [np.transpose(np.asarray(Wk, np.float32), (2, 0, 1)).reshape(C, HEADS * DK),
         np.asarray(bk, np.float32).reshape(1, HEADS * DK)], 0).astype(BF16)
    wvT = np.concatenate(
        [np.transpose(np.asarray(Wv, np.float32), (2, 0, 1)).reshape(C, HEADS * DV),
         np.asarray(bv, np.float32).reshape(1, HEADS * DV)], 0).astype(BF16)
    Wm32 = np.asarray(Wm, np.float32)
    wm0 = np.concatenate([Wm32[:, 0:DV].T,
                          np.asarray(bm, np.float32).reshape(1, C)], 0).astype(BF16)
    wm1 = Wm32[:, DV:2 * DV].T.astype(BF16)
    blk = np.zeros((128, 128), np.float32)
    blk[0:64, 0:64] = 1.0
    blk[64:128, 64:128] = 1.0
    blk = blk.astype(BF16)

    in_maps = []
    for core in range(NCORES):
        n, q = core // SHARDS_PER_N, core % SHARDS_PER_N
        qsl = slice(q * QSH, (q + 1) * QSH)
        xfb = np.concatenate([xf[n], onesL], 0).astype(BF16)
        xq = xf[n][:, qsl]
        xqb = np.concatenate([xq, np.ones((1, QSH), np.float32)], 0).astype(BF16)
        in_maps.append({
            "xfb": xfb, "xqb": xqb, "xq32": np.ascontiguousarray(xq),
            "wqT": wqT, "wkT": wkT, "wvT": wvT,
            "wm0": wm0, "wm1": wm1, "blk": blk,
        })
    return in_maps


def assemble(results):
    outf = np.empty((N, C, L), np.float32)
    for core in range(NCORES):
        n, q = core // SHARDS_PER_N, core % SHARDS_PER_N
        outf[n][:, q * QSH:(q + 1) * QSH] = results[core]["out"]
    return outf.reshape(N, C, 64, 64)


def run(inputs, trace=False, **kw):
    from concourse.bass_utils import run_bass_kernel_spmd
    nc = get_program()
    in_maps = make_in_maps(**inputs)
    res = run_bass_kernel_spmd(nc, in_maps, list(range(NCORES)), trace=trace, **kw)
    return assemble(res.results), res


def kernel(**inputs):
    out, _ = run(inputs)
    return out
